# revision 3
# baseline (speedup 1.0000x reference)
"""MoE (top-2, 8 experts) SwiGLU kernel for 8 Trainium2 NeuronCores.

Strategy (expert-parallel, per the sharding hint):
  - Host: router matmul + top-2 + softmax (tiny: [4096,1024]@[1024,8]),
    build per-expert token permutation (token-major order, capacity-truncated
    exactly like the reference's jnp.nonzero(size=CAPACITY)).
  - Host: gather each expert's tokens, transpose to [D, C], cast to bf16.
  - Device (core e == expert e): fused SwiGLU
        hT = (W_e^T x^T) * silu(V_e^T x^T)        [H, C] layout
        y  = (hT)^T-contract @ Wout_e              [D, C] layout, bf16 out
    All matmuls bf16 with fp32 PSUM accumulation; weights resident in SBUF.
  - Host: inverse-permutation gather + prob-weighted sum of the K=2 expert
    outputs per token.

Perf notes (trace-driven):
  - Head: DMA descriptors issue at ~700ns each on one engine; spread the
    first few critical descriptors across engines and split V0/x0 so the
    first matmul group starts as early as possible.
  - Mid: per-hb hT tiles so phase B's first matmuls depend only on the
    first silu/mul, not the whole chunk's worth.
  - Tail: y is written bf16 (half the outbound bytes); last chunk is the
    smallest so the post-matmul copy+DMA tail is short.
"""

import numpy as np
import ml_dtypes

import concourse.bass as bass  # noqa: F401  (bass types referenced via bacc/tile)
import concourse.mybir as mybir
import concourse.tile as tile
from concourse import bacc
from concourse.bass_utils import run_bass_kernel_spmd

B, T = 2, 2048
D_MODEL, D_HIDDEN = 1024, 2048
N_EXPERTS, TOP_K = 8, 2
N_TOKENS = B * T
CAPACITY = 2 * N_TOKENS * TOP_K // N_EXPERTS  # 2048

F32 = mybir.dt.float32
BF16 = mybir.dt.bfloat16
AF = mybir.ActivationFunctionType
BF = ml_dtypes.bfloat16

_KERNEL_CACHE: dict = {}


def _chunk_plan(C: int):
    """Chunks of <=512 with a small-ish head (fast first matmul) and the
    smallest chunk last (short tail). All matmul free dims stay >=239 so
    LDWEIGHTS (~98ns) remains hidden under the matmul stream."""
    if C <= 512:
        return [(0, C)]
    n = -(-C // 512)
    if n == 2:
        sizes = [C // 2, C - C // 2]
    else:
        head = 256
        rest = C - head
        m = n - 1
        base = rest // m
        sizes = [head] + [base + (1 if i < rest - base * m else 0)
                          for i in range(m)]
        sizes[1:] = sorted(sizes[1:], reverse=True)
    out, c0 = [], 0
    for s in sizes:
        out.append((c0, s))
        c0 += s
    assert c0 == C
    return out


def _build_expert_kernel(C: int, D: int = D_MODEL, H: int = D_HIDDEN):
    """Fused per-expert SwiGLU: y[D,C] (bf16) = ((x@W) * silu(x@V)) @ Wo."""
    assert D % 128 == 0 and H % 128 == 0
    DK, HB = D // 128, H // 128
    nc = bacc.Bacc(None, target_bir_lowering=False, debug=False)

    # xT / W / V are host-packed partition-major so every DMA segment is
    # per-partition contiguous (2 KB bursts for W/V slabs).
    xT = nc.dram_tensor("xT", [128, DK, C], BF16, kind="ExternalInput")
    Wd = nc.dram_tensor("W", [HB, 128, DK, 128], BF16, kind="ExternalInput")
    Vd = nc.dram_tensor("V", [HB, 128, DK, 128], BF16, kind="ExternalInput")
    Wo = nc.dram_tensor("Wo", [H, D], BF16, kind="ExternalInput")
    # y is produced TRANSPOSED [D, C] in bf16: tokens on the matmul free dim.
    y = nc.dram_tensor("y", [D, C], BF16, kind="ExternalOutput")

    chunks = _chunk_plan(C)
    maxc = max(cols for _, cols in chunks)

    with tile.TileContext(nc) as tc:
        with (
            tc.tile_pool(name="wpool", bufs=1) as wpool,
            tc.tile_pool(name="hpool", bufs=2) as hpool,
            tc.tile_pool(name="spool", bufs=3) as spool,
            tc.tile_pool(name="ypool", bufs=3) as ypool,
            tc.tile_pool(name="pa", bufs=2, space="PSUM") as pa_pool,
            tc.tile_pool(name="pb", bufs=2, space="PSUM") as pb_pool,
            tc.tile_pool(name="py", bufs=2, space="PSUM") as py_pool,
        ):
            x_tiles = [wpool.tile([128, DK, cols], BF16, tag=f"x{i}", name=f"x{i}")
                       for i, (_, cols) in enumerate(chunks)]
            W_tiles = [wpool.tile([128, DK, 128], BF16, tag=f"W{hb}", name=f"Wt{hb}")
                       for hb in range(HB)]
            V_tiles = [wpool.tile([128, DK, 128], BF16, tag=f"V{hb}", name=f"Vt{hb}")
                       for hb in range(HB)]
            Wo_sb = wpool.tile([128, HB, D], BF16, tag="Wo")

            xT_r = xT[:]
            Wo_r = Wo[:].rearrange("(b p) d -> p b d", p=128)

            # --- critical first bytes, spread across engine queues ---
            # First matmul group (V-branch, hb=0) needs V0 + x0; W0 follows
            # ~0.4us later. Split V0/x0/W0 in half and issue each half from a
            # different engine so descriptor-issue (~700ns each) overlaps.
            c0_0, cols_0 = chunks[0]
            h0 = DK // 2
            nc.sync.dma_start(out=V_tiles[0][:, :h0], in_=Vd[0, :, :h0])
            nc.scalar.dma_start(out=x_tiles[0][:, :h0],
                                in_=xT_r[:, :h0, c0_0:c0_0 + cols_0])
            nc.gpsimd.dma_start(out=V_tiles[0][:, h0:], in_=Vd[0, :, h0:])
            nc.sync.dma_start(out=x_tiles[0][:, h0:],
                              in_=xT_r[:, h0:, c0_0:c0_0 + cols_0])
            nc.scalar.dma_start(out=W_tiles[0][:], in_=Wd[0])
            # Remaining weight slabs in need order (V before W per hb), then
            # Wo (needed by end of chunk 0's phase A), then remaining x.
            for hb in range(1, HB):
                nc.sync.dma_start(out=V_tiles[hb][:], in_=Vd[hb])
                nc.scalar.dma_start(out=W_tiles[hb][:], in_=Wd[hb])
            nc.gpsimd.dma_start(out=Wo_sb[:], in_=Wo_r[:])
            for i, (c0, cols) in enumerate(chunks):
                if i == 0:
                    continue
                nc.gpsimd.dma_start(out=x_tiles[i][:],
                                    in_=xT_r[:, :, c0:c0 + cols])

            y_r = y[:].rearrange("(m p) c -> p m c", p=128)

            for i, (c0, cols) in enumerate(chunks):
                x_sb = x_tiles[i]
                # Per-hb hT tiles: phase B's hb-th accumulation step only
                # waits on the hb-th silu/mul, not the whole chunk.
                hts = [hpool.tile([128, maxc], BF16, tag=f"hT{hb}",
                                  name=f"hT{hb}_{i}")
                       for hb in range(HB)]
                for hb in range(HB):
                    pa = pa_pool.tile([128, maxc], F32, tag="pa")
                    pb = pb_pool.tile([128, maxc], F32, tag="pb")
                    for dk in range(DK):
                        nc.tensor.matmul(
                            pb[:, :cols], V_tiles[hb][:, dk],
                            x_sb[:, dk, :cols],
                            start=(dk == 0), stop=(dk == DK - 1),
                        )
                    for dk in range(DK):
                        nc.tensor.matmul(
                            pa[:, :cols], W_tiles[hb][:, dk],
                            x_sb[:, dk, :cols],
                            start=(dk == 0), stop=(dk == DK - 1),
                        )
                    sg = spool.tile([128, maxc], F32, tag="sg")
                    nc.scalar.activation(sg[:, :cols], pb[:, :cols], AF.Silu)
                    nc.vector.tensor_mul(hts[hb][:, :cols], pa[:, :cols],
                                         sg[:, :cols])
                for nb in range(D // 128):
                    py = py_pool.tile([128, maxc], F32, tag="py")
                    for hb in range(HB):
                        nc.tensor.matmul(
                            py[:, :cols],
                            Wo_sb[:, hb, nb * 128:(nb + 1) * 128],
                            hts[hb][:, :cols],
                            start=(hb == 0), stop=(hb == HB - 1),
                        )
                    ysb = ypool.tile([128, maxc], BF16, tag="y")
                    nc.scalar.activation(ysb[:, :cols], py[:, :cols], AF.Copy)
                    nc.sync.dma_start(out=y_r[:, nb, c0:c0 + cols],
                                      in_=ysb[:, :cols])
    nc.compile()
    return nc


def _get_kernel(C: int, D: int = D_MODEL, H: int = D_HIDDEN):
    key = (C, D, H)
    nc = _KERNEL_CACHE.get(key)
    if nc is None:
        nc = _build_expert_kernel(C, D, H)
        _KERNEL_CACHE[key] = nc
    return nc


def _router_logits(x_flat: np.ndarray, router_w: np.ndarray,
                   router_b: np.ndarray) -> np.ndarray:
    # Prefer jax-on-CPU so near-tie top-k decisions match the reference's
    # fp32 rounding as closely as possible; fall back to numpy.
    try:
        import jax
        import jax.numpy as jnp
        cpu = jax.devices("cpu")[0]
        with jax.default_device(cpu):
            lg = jnp.asarray(x_flat) @ jnp.asarray(router_w).T + jnp.asarray(router_b)
            return np.asarray(jax.device_get(lg)).astype(np.float32, copy=False)
    except Exception:
        return (x_flat @ router_w.T + router_b).astype(np.float32)


def kernel(x, router_w, router_b, W, V, W_out):
    Bq, Tq, D = x.shape
    N = Bq * Tq
    x_flat = np.ascontiguousarray(x, dtype=np.float32).reshape(N, D)

    # ---- routing (host) ----
    logits = _router_logits(x_flat, router_w, router_b)          # [N, E]
    order2 = np.argsort(-logits, axis=1, kind="stable")[:, :TOP_K]  # lax.top_k ties
    top_ids = order2.astype(np.int64)                            # [N, K]
    top_vals = np.take_along_axis(logits, top_ids, axis=1)
    mx = top_vals.max(axis=1, keepdims=True)
    ex = np.exp((top_vals - mx).astype(np.float32))
    probs = (ex / ex.sum(axis=1, keepdims=True)).astype(np.float32)

    # ---- permutation (token-major scan order, capacity truncation) ----
    flat_e = top_ids.ravel()                                     # [N*K]
    scan = np.argsort(flat_e, kind="stable")                     # grouped by expert
    counts = np.bincount(flat_e, minlength=N_EXPERTS)
    starts = np.zeros(N_EXPERTS + 1, dtype=np.int64)
    starts[1:] = np.cumsum(counts)
    C = int(min(CAPACITY, max(counts.max(), 1)))  # exact compute width

    tok_pad = np.full((N_EXPERTS, C), N, dtype=np.int64)
    slot_pad = np.zeros((N_EXPERTS, C), dtype=np.int64)
    pos_of_pair = np.full(N * TOP_K, -1, dtype=np.int64)
    for e in range(N_EXPERTS):
        idxs = scan[starts[e]:starts[e + 1]][:C]
        tok_pad[e, :len(idxs)] = idxs // TOP_K
        slot_pad[e, :len(idxs)] = idxs % TOP_K
        pos_of_pair[idxs] = e * C + np.arange(len(idxs))

    # ---- per-core device inputs ----
    x_pad = np.vstack([x_flat, np.zeros((1, D), np.float32)])
    probs_pad = np.vstack([probs, np.zeros((1, TOP_K), np.float32)])
    def _pack(mat):  # [D, H] -> [HB, 128, DK, 128] partition-major slabs
        Dm, Hm = mat.shape
        return np.ascontiguousarray(
            mat.astype(BF).reshape(Dm // 128, 128, Hm // 128, 128)
            .transpose(2, 1, 0, 3))

    in_maps = []
    w_scales = []
    for e in range(N_EXPERTS):
        xg = x_pad[tok_pad[e]]                                   # [C, D]
        w_e = probs_pad[tok_pad[e], slot_pad[e]].astype(np.float32)  # [C]
        xTp = (xg.T.astype(BF)                                   # [D, C] ->
               .reshape(D // 128, 128, C).transpose(1, 0, 2))    # [128, DK, C]
        in_maps.append({
            "xT": np.ascontiguousarray(xTp),
            "W": _pack(W[e]),
            "V": _pack(V[e]),
            "Wo": W_out[e].astype(BF),
        })
        w_scales.append(w_e)

    # ---- run on 8 cores ----
    H = W.shape[2]
    nc = _get_kernel(C, D, H)
    res = None
    for attempt in range(2):
        try:
            res = run_bass_kernel_spmd(nc, in_maps,
                                       core_ids=list(range(N_EXPERTS)))
            break
        except Exception as err:  # transient axon/device errors: retry once
            import sys
            print(f"kernel: device run attempt {attempt} failed: {err!r}",
                  file=sys.stderr)
    if res is not None:
        y_list = [res.results[e]["y"].astype(np.float32).T * w_scales[e][:, None]
                  for e in range(N_EXPERTS)]
    else:  # last resort so a flaky device doesn't turn into a crash
        import sys
        print("kernel: falling back to host compute", file=sys.stderr)
        y_list = []
        for e in range(N_EXPERTS):
            xg = x_pad[tok_pad[e]]
            a = xg @ W[e]
            b = xg @ V[e]
            yy = (a * (b / (1.0 + np.exp(-b)))) @ W_out[e]
            w_e = probs_pad[tok_pad[e], slot_pad[e]][:, None]
            y_list.append((yy * w_e).astype(np.float32))
    y_all = np.concatenate(y_list, axis=0)                       # [E*C, D]
    y_all = np.vstack([y_all, np.zeros((1, D), np.float32)])     # drop row

    # ---- combine (host): out[n] = sum_k y_scaled[expert_k(n), pos_k(n)] ----
    pos = np.where(pos_of_pair < 0, N_EXPERTS * C, pos_of_pair)
    out_flat = y_all[pos].reshape(N, TOP_K, D).sum(axis=1)
    return out_flat.reshape(Bq, Tq, D).astype(np.float32, copy=False)


# revision 5
# speedup vs baseline: 1.2107x; 1.2107x over previous
"""MoE (top-2, 8 experts) SwiGLU kernel for 8 Trainium2 NeuronCores.

Strategy (expert-parallel, per the sharding hint):
  - Host: router matmul + top-2 + softmax (tiny: [4096,1024]@[1024,8]),
    build per-expert token permutation (token-major order, capacity-truncated
    exactly like the reference's jnp.nonzero(size=CAPACITY)).
  - Host: gather each expert's tokens, transpose to [D, C], cast to bf16.
  - Device (core e == expert e): fused SwiGLU
        hT = (W_e^T x^T) * silu(V_e^T x^T)        [H, C] layout
        y  = (hT)^T-contract @ Wout_e              [D, C] layout, bf16 out
    All matmuls bf16 with fp32 PSUM accumulation; weights resident in SBUF.
  - Host: inverse-permutation gather + prob-weighted sum of the K=2 expert
    outputs per token.

Perf notes (trace-driven):
  - Head: DMA descriptors issue at ~700ns each on one engine; spread the
    first few critical descriptors across engines and split V0/x0 so the
    first matmul group starts as early as possible.
  - Mid: per-hb hT tiles so phase B's first matmuls depend only on the
    first silu/mul, not the whole chunk's worth.
  - Tail: y is written bf16 (half the outbound bytes); last chunk is the
    smallest so the post-matmul copy+DMA tail is short.
"""

import numpy as np
import ml_dtypes

import concourse.bass as bass  # noqa: F401  (bass types referenced via bacc/tile)
import concourse.mybir as mybir
import concourse.tile as tile
from concourse import bacc
from concourse.bass_utils import run_bass_kernel_spmd

B, T = 2, 2048
D_MODEL, D_HIDDEN = 1024, 2048
N_EXPERTS, TOP_K = 8, 2
N_TOKENS = B * T
CAPACITY = 2 * N_TOKENS * TOP_K // N_EXPERTS  # 2048

F32 = mybir.dt.float32
BF16 = mybir.dt.bfloat16
AF = mybir.ActivationFunctionType
BF = ml_dtypes.bfloat16

_KERNEL_CACHE: dict = {}


def _chunk_plan(C: int):
    """Near-equal chunks of <=512. Chunk 0 must be big enough that phase A
    (which consumes one 524KB W/V slab pair per 16 matmuls) does not outrun
    the ~350GB/s DMA stream: N>=384 keeps the demand at ~205GB/s."""
    n = max(1, -(-C // 512))
    base = C // n
    sizes = sorted((base + (1 if i < C - base * n else 0) for i in range(n)),
                   reverse=True)
    out, c0 = [], 0
    for s in sizes:
        out.append((c0, s))
        c0 += s
    assert c0 == C
    return out


def _build_expert_kernel(C: int, D: int = D_MODEL, H: int = D_HIDDEN):
    """Fused per-expert SwiGLU: y[D,C] (bf16) = ((x@W) * silu(x@V)) @ Wo."""
    assert D % 128 == 0 and H % 128 == 0
    DK, HB = D // 128, H // 128
    nc = bacc.Bacc(None, target_bir_lowering=False, debug=False)

    # xT / W / V are host-packed partition-major so every DMA segment is
    # per-partition contiguous (2 KB bursts for W/V slabs).
    xT = nc.dram_tensor("xT", [128, DK, C], BF16, kind="ExternalInput")
    Wd = nc.dram_tensor("W", [HB, 128, DK, 128], BF16, kind="ExternalInput")
    Vd = nc.dram_tensor("V", [HB, 128, DK, 128], BF16, kind="ExternalInput")
    Wo = nc.dram_tensor("Wo", [H, D], BF16, kind="ExternalInput")
    # y is produced TRANSPOSED [D, C] in bf16: tokens on the matmul free dim.
    y = nc.dram_tensor("y", [D, C], BF16, kind="ExternalOutput")

    chunks = _chunk_plan(C)
    maxc = max(cols for _, cols in chunks)

    with tile.TileContext(nc) as tc:
        with (
            tc.tile_pool(name="wpool", bufs=1) as wpool,
            tc.tile_pool(name="hpool", bufs=2) as hpool,
            tc.tile_pool(name="spool", bufs=3) as spool,
            tc.tile_pool(name="ypool", bufs=3) as ypool,
            tc.tile_pool(name="pa", bufs=2, space="PSUM") as pa_pool,
            tc.tile_pool(name="pb", bufs=2, space="PSUM") as pb_pool,
            tc.tile_pool(name="py", bufs=2, space="PSUM") as py_pool,
        ):
            x_tiles = [wpool.tile([128, DK, cols], BF16, tag=f"x{i}", name=f"x{i}")
                       for i, (_, cols) in enumerate(chunks)]
            W_tiles = [wpool.tile([128, DK, 128], BF16, tag=f"W{hb}", name=f"Wt{hb}")
                       for hb in range(HB)]
            V_tiles = [wpool.tile([128, DK, 128], BF16, tag=f"V{hb}", name=f"Vt{hb}")
                       for hb in range(HB)]
            Wo_sb = wpool.tile([128, HB, D], BF16, tag="Wo")

            xT_r = xT[:]
            Wo_r = Wo[:].rearrange("(b p) d -> p b d", p=128)

            # --- critical first bytes, spread across engine queues ---
            # First matmul group (V-branch, hb=0) needs V0 + x0; W0 follows
            # ~0.4us later. Split V0/x0/W0 in half and issue each half from a
            # different engine so descriptor-issue (~700ns each) overlaps.
            # Critical first bytes on three parallel queues (the queues
            # round-robin packets over the same 16 engines, so concurrency
            # only helps while the head descriptors are genuinely parallel):
            #   sync:   V0 lo-half, then the whole in-order weight stream
            #   scalar: x0 lo-half then hi-half (first matmul group input)
            #   gpsimd: V0 hi-half + W0, then nothing until the x tail
            c0_0, cols_0 = chunks[0]
            h0 = DK // 2
            nc.sync.dma_start(out=V_tiles[0][:, :h0], in_=Vd[0, :, :h0])
            nc.scalar.dma_start(out=x_tiles[0][:, :h0],
                                in_=xT_r[:, :h0, c0_0:c0_0 + cols_0])
            nc.gpsimd.dma_start(out=V_tiles[0][:, h0:], in_=Vd[0, :, h0:])
            nc.scalar.dma_start(out=x_tiles[0][:, h0:],
                                in_=xT_r[:, h0:, c0_0:c0_0 + cols_0])
            nc.gpsimd.dma_start(out=W_tiles[0][:], in_=Wd[0])
            # Remaining stream strictly in consumption order on ONE queue:
            # W/V slabs (phase A), Wo (phase B of chunk 0), remaining x.
            for hb in range(1, HB):
                nc.sync.dma_start(out=V_tiles[hb][:], in_=Vd[hb])
                nc.sync.dma_start(out=W_tiles[hb][:], in_=Wd[hb])
            nc.sync.dma_start(out=Wo_sb[:], in_=Wo_r[:])
            for i, (c0, cols) in enumerate(chunks):
                if i == 0:
                    continue
                nc.sync.dma_start(out=x_tiles[i][:],
                                  in_=xT_r[:, :, c0:c0 + cols])

            y_r = y[:].rearrange("(m p) c -> p m c", p=128)

            for i, (c0, cols) in enumerate(chunks):
                x_sb = x_tiles[i]
                # Per-hb hT tiles: phase B's hb-th accumulation step only
                # waits on the hb-th silu/mul, not the whole chunk.
                hts = [hpool.tile([128, maxc], BF16, tag=f"hT{hb}",
                                  name=f"hT{hb}_{i}")
                       for hb in range(HB)]
                for hb in range(HB):
                    pa = pa_pool.tile([128, maxc], F32, tag="pa")
                    pb = pb_pool.tile([128, maxc], F32, tag="pb")
                    for dk in range(DK):
                        nc.tensor.matmul(
                            pb[:, :cols], V_tiles[hb][:, dk],
                            x_sb[:, dk, :cols],
                            start=(dk == 0), stop=(dk == DK - 1),
                        )
                    for dk in range(DK):
                        nc.tensor.matmul(
                            pa[:, :cols], W_tiles[hb][:, dk],
                            x_sb[:, dk, :cols],
                            start=(dk == 0), stop=(dk == DK - 1),
                        )
                    sg = spool.tile([128, maxc], F32, tag="sg")
                    nc.scalar.activation(sg[:, :cols], pb[:, :cols], AF.Silu)
                    nc.vector.tensor_mul(hts[hb][:, :cols], pa[:, :cols],
                                         sg[:, :cols])
                for nb in range(D // 128):
                    py = py_pool.tile([128, maxc], F32, tag="py")
                    for hb in range(HB):
                        nc.tensor.matmul(
                            py[:, :cols],
                            Wo_sb[:, hb, nb * 128:(nb + 1) * 128],
                            hts[hb][:, :cols],
                            start=(hb == 0), stop=(hb == HB - 1),
                        )
                    ysb = ypool.tile([128, maxc], BF16, tag="y")
                    nc.scalar.activation(ysb[:, :cols], py[:, :cols], AF.Copy)
                    nc.sync.dma_start(out=y_r[:, nb, c0:c0 + cols],
                                      in_=ysb[:, :cols])
    nc.compile()
    return nc


def _get_kernel(C: int, D: int = D_MODEL, H: int = D_HIDDEN):
    key = (C, D, H)
    nc = _KERNEL_CACHE.get(key)
    if nc is None:
        nc = _build_expert_kernel(C, D, H)
        _KERNEL_CACHE[key] = nc
    return nc


def _router_logits(x_flat: np.ndarray, router_w: np.ndarray,
                   router_b: np.ndarray) -> np.ndarray:
    # Prefer jax-on-CPU so near-tie top-k decisions match the reference's
    # fp32 rounding as closely as possible; fall back to numpy.
    try:
        import jax
        import jax.numpy as jnp
        cpu = jax.devices("cpu")[0]
        with jax.default_device(cpu):
            lg = jnp.asarray(x_flat) @ jnp.asarray(router_w).T + jnp.asarray(router_b)
            return np.asarray(jax.device_get(lg)).astype(np.float32, copy=False)
    except Exception:
        return (x_flat @ router_w.T + router_b).astype(np.float32)


def kernel(x, router_w, router_b, W, V, W_out):
    Bq, Tq, D = x.shape
    N = Bq * Tq
    x_flat = np.ascontiguousarray(x, dtype=np.float32).reshape(N, D)

    # ---- routing (host) ----
    logits = _router_logits(x_flat, router_w, router_b)          # [N, E]
    order2 = np.argsort(-logits, axis=1, kind="stable")[:, :TOP_K]  # lax.top_k ties
    top_ids = order2.astype(np.int64)                            # [N, K]
    top_vals = np.take_along_axis(logits, top_ids, axis=1)
    mx = top_vals.max(axis=1, keepdims=True)
    ex = np.exp((top_vals - mx).astype(np.float32))
    probs = (ex / ex.sum(axis=1, keepdims=True)).astype(np.float32)

    # ---- permutation (token-major scan order, capacity truncation) ----
    flat_e = top_ids.ravel()                                     # [N*K]
    scan = np.argsort(flat_e, kind="stable")                     # grouped by expert
    counts = np.bincount(flat_e, minlength=N_EXPERTS)
    starts = np.zeros(N_EXPERTS + 1, dtype=np.int64)
    starts[1:] = np.cumsum(counts)
    C = int(min(CAPACITY, max(counts.max(), 1)))  # exact compute width

    tok_pad = np.full((N_EXPERTS, C), N, dtype=np.int64)
    slot_pad = np.zeros((N_EXPERTS, C), dtype=np.int64)
    pos_of_pair = np.full(N * TOP_K, -1, dtype=np.int64)
    for e in range(N_EXPERTS):
        idxs = scan[starts[e]:starts[e + 1]][:C]
        tok_pad[e, :len(idxs)] = idxs // TOP_K
        slot_pad[e, :len(idxs)] = idxs % TOP_K
        pos_of_pair[idxs] = e * C + np.arange(len(idxs))

    # ---- per-core device inputs ----
    x_pad = np.vstack([x_flat, np.zeros((1, D), np.float32)])
    probs_pad = np.vstack([probs, np.zeros((1, TOP_K), np.float32)])
    def _pack(mat):  # [D, H] -> [HB, 128, DK, 128] partition-major slabs
        Dm, Hm = mat.shape
        return np.ascontiguousarray(
            mat.astype(BF).reshape(Dm // 128, 128, Hm // 128, 128)
            .transpose(2, 1, 0, 3))

    in_maps = []
    w_scales = []
    for e in range(N_EXPERTS):
        xg = x_pad[tok_pad[e]]                                   # [C, D]
        w_e = probs_pad[tok_pad[e], slot_pad[e]].astype(np.float32)  # [C]
        xTp = (xg.T.astype(BF)                                   # [D, C] ->
               .reshape(D // 128, 128, C).transpose(1, 0, 2))    # [128, DK, C]
        in_maps.append({
            "xT": np.ascontiguousarray(xTp),
            "W": _pack(W[e]),
            "V": _pack(V[e]),
            "Wo": W_out[e].astype(BF),
        })
        w_scales.append(w_e)

    # ---- run on 8 cores ----
    H = W.shape[2]
    nc = _get_kernel(C, D, H)
    res = None
    for attempt in range(2):
        try:
            res = run_bass_kernel_spmd(nc, in_maps,
                                       core_ids=list(range(N_EXPERTS)))
            break
        except Exception as err:  # transient axon/device errors: retry once
            import sys
            print(f"kernel: device run attempt {attempt} failed: {err!r}",
                  file=sys.stderr)
    if res is not None:
        y_list = [res.results[e]["y"].astype(np.float32).T * w_scales[e][:, None]
                  for e in range(N_EXPERTS)]
    else:  # last resort so a flaky device doesn't turn into a crash
        import sys
        print("kernel: falling back to host compute", file=sys.stderr)
        y_list = []
        for e in range(N_EXPERTS):
            xg = x_pad[tok_pad[e]]
            a = xg @ W[e]
            b = xg @ V[e]
            yy = (a * (b / (1.0 + np.exp(-b)))) @ W_out[e]
            w_e = probs_pad[tok_pad[e], slot_pad[e]][:, None]
            y_list.append((yy * w_e).astype(np.float32))
    y_all = np.concatenate(y_list, axis=0)                       # [E*C, D]
    y_all = np.vstack([y_all, np.zeros((1, D), np.float32)])     # drop row

    # ---- combine (host): out[n] = sum_k y_scaled[expert_k(n), pos_k(n)] ----
    pos = np.where(pos_of_pair < 0, N_EXPERTS * C, pos_of_pair)
    out_flat = y_all[pos].reshape(N, TOP_K, D).sum(axis=1)
    return out_flat.reshape(Bq, Tq, D).astype(np.float32, copy=False)


# revision 6
# speedup vs baseline: 1.2114x; 1.0006x over previous
"""MoE (top-2, 8 experts) SwiGLU kernel for 8 Trainium2 NeuronCores.

v3: slot-balanced expert×H/4 jobs.

Each expert's SwiGLU is split along the hidden dim into 4 jobs of
H/4 = 512 hidden units (4 slabs of 128). The 32 jobs, each costing
C_e tokens, are sorted by size and dealt into 4 "slots" of 8 jobs;
core k runs slot s's k-th job. The compiled program processes the 4
slots at fixed sizes S[s] = max count in slot s, which for near-uniform
routing gives an effective per-core token count of mean(S) ~= 1030
instead of max_e C_e ~= 1071.

Per slot the device computes a partial SwiGLU:
    hT = (Wq^T x^T) * silu(Vq^T x^T)      [H/4, S] layout
    y_s = hT^T-contract @ Wo_q            [D, S] bf16 partial
and the host sums the 4 partials of each expert, applies routing probs,
and scatters back.
"""

import numpy as np
import ml_dtypes

import concourse.bass as bass  # noqa: F401
import concourse.mybir as mybir
import concourse.tile as tile
from concourse import bacc
from concourse.bass_utils import run_bass_kernel_spmd

B, T = 2, 2048
D_MODEL, D_HIDDEN = 1024, 2048
N_EXPERTS, TOP_K = 8, 2
N_TOKENS = B * T
CAPACITY = 2 * N_TOKENS * TOP_K // N_EXPERTS  # 2048

NSLOT = 4                      # jobs per core == H splits per expert
HBJ = D_HIDDEN // 128 // NSLOT  # hb slabs per job (4)

F32 = mybir.dt.float32
BF16 = mybir.dt.bfloat16
AF = mybir.ActivationFunctionType
BF = ml_dtypes.bfloat16

_KERNEL_CACHE: dict = {}


def _pad_slot(C: int) -> int:
    """Pad a slot size to a multiple of 8 tokens (16B SBUF alignment for
    the per-dk x slices; misaligned moving operands cost ~26ns/matmul)."""
    return -(-C // 8) * 8


def _chunk_plan(C: int):
    """Near-equal multiple-of-8 chunks of <=512 (PSUM bank limit)."""
    assert C % 8 == 0
    n = max(1, -(-C // 512))
    k = C // 8
    base = k // n * 8
    rem = (C - base * n) // 8
    sizes = sorted((base + 8 * (1 if i < rem else 0) for i in range(n)),
                   reverse=True)
    out, c0 = [], 0
    for s in sizes:
        out.append((c0, s))
        c0 += s
    assert c0 == C
    return out


def _build_kernel(S: tuple, D: int = D_MODEL, H: int = D_HIDDEN):
    """4-slot partial-SwiGLU program; slot s processes S[s] tokens with
    H/4 hidden units."""
    DK = D // 128
    nc = bacc.Bacc(None, target_bir_lowering=False, debug=False)

    plans = [_chunk_plan(Sz) for Sz in S]
    maxc = max(cols for plan in plans for _, cols in plan)

    # x arrives chunk-contiguous: [128, DK*cols] per chunk, so every DMA
    # segment is a full per-partition run (2KB packets, not 714B strips).
    xTs, Wds, Vds, Wos, ys = [], [], [], [], []
    for s, Sz in enumerate(S):
        xTs.append(nc.dram_tensor(f"xT{s}", [128, DK * Sz], BF16,
                                  kind="ExternalInput"))
        Wds.append(nc.dram_tensor(f"W{s}", [HBJ, 128, DK, 128], BF16,
                                  kind="ExternalInput"))
        Vds.append(nc.dram_tensor(f"V{s}", [HBJ, 128, DK, 128], BF16,
                                  kind="ExternalInput"))
        Wos.append(nc.dram_tensor(f"Wo{s}", [HBJ, 128, D], BF16,
                                  kind="ExternalInput"))
        # y is chunk-major [128, 8*Sz]: chunk i's nb-block at column
        # 8*c0 + nb*cols, so paired-nb DMAs write contiguous 2KB runs.
        ys.append(nc.dram_tensor(f"y{s}", [128, (D // 128) * Sz], BF16,
                                 kind="ExternalOutput"))

    with tile.TileContext(nc) as tc:
        with (
            tc.tile_pool(name="wpool", bufs=1) as wpool,
            tc.tile_pool(name="hpool", bufs=2) as hpool,
            tc.tile_pool(name="spool", bufs=3) as spool,
            tc.tile_pool(name="ypool", bufs=4) as ypool,
            tc.tile_pool(name="pa", bufs=2, space="PSUM") as pa_pool,
            tc.tile_pool(name="pb", bufs=2, space="PSUM") as pb_pool,
            tc.tile_pool(name="py", bufs=4, space="PSUM") as py_pool,
        ):
            # All four slots' weights stay resident (4*HBJ = 16 hb slabs
            # total, same SBUF budget as one full expert).
            W_tiles = [[wpool.tile([128, DK, 128], BF16, tag=f"W{s}_{j}",
                                   name=f"Wt{s}_{j}") for j in range(HBJ)]
                       for s in range(NSLOT)]
            V_tiles = [[wpool.tile([128, DK, 128], BF16, tag=f"V{s}_{j}",
                                   name=f"Vt{s}_{j}") for j in range(HBJ)]
                       for s in range(NSLOT)]
            Wo_sb = [wpool.tile([128, HBJ, D], BF16, tag=f"Wo{s}", name=f"Wo{s}")
                     for s in range(NSLOT)]

            # x tiles: per (slot, chunk), all resident (~66KB/partition).
            x_tiles = {}
            for s in range(NSLOT):
                for i, (c0, cols) in enumerate(plans[s]):
                    x_tiles[(s, i)] = wpool.tile(
                        [128, DK, cols], BF16, tag=f"x{s}_{i}",
                        name=f"x{s}_{i}")

            # ---- PE warmup ----
            # ~20 junk matmuls on zeroed scratch keep the PE busy through
            # the HAM SHORT window (~3.4us) while the first DMAs stream in,
            # so the real matmuls start at 2.4GHz instead of 1.2GHz.
            # Memsets go on the VECTOR engine (free by ~4.2us; gpsimd is
            # busy with its own init memsets until ~6us).
            warm_w = wpool.tile([128, 128], BF16, tag="warmw", name="warm_w")
            warm_x = wpool.tile([128, 512], BF16, tag="warmx", name="warm_x")
            nc.vector.memzero(warm_w[:])
            nc.vector.memzero(warm_x[:])
            warm_p = pa_pool.tile([128, maxc], F32, tag="pa", name="warm_p")
            for _ in range(20):
                nc.tensor.matmul(warm_p[:, :512], warm_w[:], warm_x[:],
                                 start=True, stop=True)

            # ---- DMA issue: ONE queue, strictly in consumption order ----
            # (parallel queues round-robin over the same 16 engines and
            # dilute the weight-slab stream -> phase A starvation).
            def _x_dma(s, i):
                c0, cols = plans[s][i]
                nc.sync.dma_start(
                    out=x_tiles[(s, i)][:].rearrange("p a b -> p (a b)"),
                    in_=xTs[s][:][:, DK * c0:DK * (c0 + cols)])

            # slot 0 head: dk-halved V0/x00/W0 so the first matmul group
            # starts after ~0.3MB instead of ~1.3MB.
            h0 = DK // 2
            c00, cols00 = plans[0][0]
            x00 = x_tiles[(0, 0)]
            nc.sync.dma_start(out=V_tiles[0][0][:, :h0], in_=Vds[0][0, :, :h0])
            nc.sync.dma_start(out=x00[:, :h0].rearrange("p a b -> p (a b)"),
                              in_=xTs[0][:][:, :h0 * cols00])
            nc.sync.dma_start(out=V_tiles[0][0][:, h0:], in_=Vds[0][0, :, h0:])
            nc.sync.dma_start(out=x00[:, h0:].rearrange("p a b -> p (a b)"),
                              in_=xTs[0][:][:, h0 * cols00:DK * cols00])
            nc.sync.dma_start(out=W_tiles[0][0][:, :h0], in_=Wds[0][0, :, :h0])
            nc.sync.dma_start(out=W_tiles[0][0][:, h0:], in_=Wds[0][0, :, h0:])
            for s in range(NSLOT):
                if s == 0:
                    pass
                else:
                    nc.sync.dma_start(out=V_tiles[s][0][:], in_=Vds[s][0])
                    nc.sync.dma_start(out=W_tiles[s][0][:], in_=Wds[s][0])
                for j in range(1, HBJ):
                    nc.sync.dma_start(out=V_tiles[s][j][:], in_=Vds[s][j])
                    nc.sync.dma_start(out=W_tiles[s][j][:], in_=Wds[s][j])
                nc.sync.dma_start(out=Wo_sb[s][:],
                                  in_=Wos[s][:].rearrange("b p d -> p b d"))
                for i in range(len(plans[s])):
                    if (s, i) != (0, 0):
                        _x_dma(s, i)

            # ---- compute ----
            NB = D // 128
            for s in range(NSLOT):
                y_flat = ys[s][:]
                for i, (c0, cols) in enumerate(plans[s]):
                    x_sb = x_tiles[(s, i)]
                    hts = [hpool.tile([128, maxc], BF16, tag=f"hT{j}",
                                      name=f"hT{s}_{i}_{j}")
                           for j in range(HBJ)]
                    for j in range(HBJ):
                        pa = pa_pool.tile([128, maxc], F32, tag="pa", name="pa")
                        pb = pb_pool.tile([128, maxc], F32, tag="pb", name="pb")
                        for dk in range(DK):
                            nc.tensor.matmul(
                                pb[:, :cols], V_tiles[s][j][:, dk],
                                x_sb[:, dk, :cols],
                                start=(dk == 0), stop=(dk == DK - 1),
                            )
                        for dk in range(DK):
                            nc.tensor.matmul(
                                pa[:, :cols], W_tiles[s][j][:, dk],
                                x_sb[:, dk, :cols],
                                start=(dk == 0), stop=(dk == DK - 1),
                            )
                        sg = spool.tile([128, maxc], F32, tag="sg", name="sg")
                        nc.scalar.activation(sg[:, :cols], pb[:, :cols],
                                             AF.Silu)
                        nc.vector.tensor_mul(hts[j][:, :cols], pa[:, :cols],
                                             sg[:, :cols])
                    for nb in range(NB):
                        py = py_pool.tile([128, maxc], F32, tag="py", name="py")
                        for j in range(HBJ):
                            nc.tensor.matmul(
                                py[:, :cols],
                                Wo_sb[s][:, j, nb * 128:(nb + 1) * 128],
                                hts[j][:, :cols],
                                start=(j == 0), stop=(j == HBJ - 1),
                            )
                        # Copy runs on the (mostly idle) VECTOR engine; on
                        # scalar it queues behind silus/other copies and the
                        # whole phase-B loop becomes scalar-paced.
                        if nb % 2 == 0:
                            ysb = ypool.tile([128, 2, maxc], BF16, tag="y", name="ysb")
                        nc.vector.tensor_copy(ysb[:, nb % 2, :cols],
                                              py[:, :cols])
                        if nb % 2 == 1:
                            # y goes out on the gpsimd queue: its completion
                            # semaphores must not alias the input stream's
                            # (a shared sem once stalled slot N+1's first
                            # matmul 9us behind slot N's y flush).
                            off = NB * c0 + (nb - 1) * cols
                            nc.gpsimd.dma_start(
                                out=y_flat[:, off:off + 2 * cols].rearrange(
                                    "p (a b) -> p a b", a=2),
                                in_=ysb[:, :, :cols])
    nc.compile()
    return nc


def _get_kernel(S: tuple):
    nc = _KERNEL_CACHE.get(S)
    if nc is None:
        nc = _build_kernel(S)
        _KERNEL_CACHE[S] = nc
    return nc


def _router_logits(x_flat, router_w, router_b):
    try:
        import jax
        import jax.numpy as jnp
        cpu = jax.devices("cpu")[0]
        with jax.default_device(cpu):
            lg = jnp.asarray(x_flat) @ jnp.asarray(router_w).T + jnp.asarray(router_b)
            return np.asarray(jax.device_get(lg)).astype(np.float32, copy=False)
    except Exception:
        return (x_flat @ router_w.T + router_b).astype(np.float32)


def kernel(x, router_w, router_b, W, V, W_out):
    Bq, Tq, D = x.shape
    N = Bq * Tq
    H = W.shape[2]
    x_flat = np.ascontiguousarray(x, dtype=np.float32).reshape(N, D)

    # ---- routing (host) ----
    logits = _router_logits(x_flat, router_w, router_b)
    order2 = np.argsort(-logits, axis=1, kind="stable")[:, :TOP_K]
    top_ids = order2.astype(np.int64)
    top_vals = np.take_along_axis(logits, top_ids, axis=1)
    mx = top_vals.max(axis=1, keepdims=True)
    ex = np.exp((top_vals - mx).astype(np.float32))
    probs = (ex / ex.sum(axis=1, keepdims=True)).astype(np.float32)

    # ---- permutation ----
    flat_e = top_ids.ravel()
    scan = np.argsort(flat_e, kind="stable")
    counts = np.bincount(flat_e, minlength=N_EXPERTS)
    starts = np.zeros(N_EXPERTS + 1, dtype=np.int64)
    starts[1:] = np.cumsum(counts)
    Ce = np.minimum(counts, CAPACITY).astype(np.int64)     # per-expert count

    tok = np.full((N_EXPERTS, int(Ce.max())), N, dtype=np.int64)
    slot_idx = np.zeros((N_EXPERTS, int(Ce.max())), dtype=np.int64)
    pos_of_pair = np.full(N * TOP_K, -1, dtype=np.int64)
    Cmax = int(Ce.max())
    for e in range(N_EXPERTS):
        idxs = scan[starts[e]:starts[e + 1]][:Cmax]
        tok[e, :len(idxs)] = idxs // TOP_K
        slot_idx[e, :len(idxs)] = idxs % TOP_K
        pos_of_pair[idxs] = e * Cmax + np.arange(len(idxs))

    # ---- job schedule: 4 H-jobs per expert, dealt into 4 slots of 8 ----
    jobs = sorted(((int(Ce[e]), e, q) for e in range(N_EXPERTS)
                   for q in range(NSLOT)), reverse=True)
    S = tuple(_pad_slot(jobs[8 * s][0]) for s in range(NSLOT))  # slot max, padded
    assign = {}                                             # (core, slot) -> (e, q)
    for s in range(NSLOT):
        for k in range(8):
            _, e, q = jobs[8 * s + k]
            assign[(k, s)] = (e, q)

    # ---- per-core device inputs ----
    x_pad = np.vstack([x_flat, np.zeros((1, D), np.float32)])
    probs_pad = np.vstack([probs, np.zeros((1, TOP_K), np.float32)])

    def _pack(mat):  # [D, H'] -> [HB', 128, DK, 128]
        Dm, Hm = mat.shape
        return np.ascontiguousarray(
            mat.astype(BF).reshape(Dm // 128, 128, Hm // 128, 128)
            .transpose(2, 1, 0, 3))

    xT_e, Wp_e, Vp_e, Wop_e, w_sc = {}, {}, {}, {}, {}
    for e in range(N_EXPERTS):
        ce = int(Ce[e])
        xg = x_pad[tok[e, :ce]]                              # [ce, D]
        xT_e[e] = np.ascontiguousarray(
            xg.T.astype(BF).reshape(D // 128, 128, ce).transpose(1, 0, 2))
        Wp_e[e] = _pack(W[e])                                # [HB,128,DK,128]
        Vp_e[e] = _pack(V[e])
        Wop_e[e] = np.ascontiguousarray(                     # [HB, 128, D]
            W_out[e].astype(BF).reshape(H // 128, 128, D))
        w_sc[e] = probs_pad[tok[e, :ce], slot_idx[e, :ce]].astype(np.float32)

    DK = D // 128
    # chunk-contiguous x, built once per (expert, slot-size)
    xflat_cache = {}

    def _xflat(e, Sz):
        key = (e, Sz)
        if key not in xflat_cache:
            ce = int(Ce[e])
            xp = np.zeros((128, DK, Sz), dtype=BF)
            xp[:, :, :ce] = xT_e[e]
            out = np.empty((128, DK * Sz), dtype=BF)
            for c0, cols in _chunk_plan(Sz):
                out[:, DK * c0:DK * (c0 + cols)] = \
                    xp[:, :, c0:c0 + cols].reshape(128, -1)
            xflat_cache[key] = out
        return xflat_cache[key]

    in_maps = []
    for k in range(8):
        m = {}
        for s in range(NSLOT):
            e, q = assign[(k, s)]
            m[f"xT{s}"] = _xflat(e, S[s])
            m[f"W{s}"] = Wp_e[e][HBJ * q:HBJ * (q + 1)]
            m[f"V{s}"] = Vp_e[e][HBJ * q:HBJ * (q + 1)]
            m[f"Wo{s}"] = np.ascontiguousarray(
                Wop_e[e][HBJ * q:HBJ * (q + 1)])
        in_maps.append(m)

    # ---- run on 8 cores ----
    nc = _get_kernel(S)
    res = None
    for attempt in range(2):
        try:
            res = run_bass_kernel_spmd(nc, in_maps, core_ids=list(range(8)))
            break
        except Exception as err:
            import sys
            print(f"kernel: device run attempt {attempt} failed: {err!r}",
                  file=sys.stderr)

    y_e = {}
    if res is not None:
        NBq = D // 128
        for s in range(NSLOT):
            Sz = S[s]
            plan = _chunk_plan(Sz)
            for k in range(8):
                e, q = assign[(k, s)]
                ce = int(Ce[e])
                ydev = res.results[k][f"y{s}"]          # [128, 8*Sz] bf16
                part = np.empty((D, ce), np.float32)
                for c0, cols in plan:
                    if c0 >= ce:
                        continue
                    take = min(cols, ce - c0)
                    blk = (ydev[:, NBq * c0:NBq * (c0 + cols)]
                           .reshape(128, NBq, cols)[:, :, :take]
                           .astype(np.float32))
                    part[:, c0:c0 + take] = \
                        blk.transpose(1, 0, 2).reshape(D, take)
                y_e[e] = part if e not in y_e else y_e[e] + part
        y_list = [(y_e[e].T * w_sc[e][:, None]) for e in range(N_EXPERTS)]
    else:
        import sys
        print("kernel: falling back to host compute", file=sys.stderr)
        y_list = []
        for e in range(N_EXPERTS):
            ce = int(Ce[e])
            xg = x_pad[tok[e, :ce]]
            a = xg @ W[e]
            b = xg @ V[e]
            yy = (a * (b / (1.0 + np.exp(-b)))) @ W_out[e]
            y_list.append((yy * w_sc[e][:, None]).astype(np.float32))

    # pad each expert's block to Cmax for the positional gather
    y_all = np.zeros((N_EXPERTS * Cmax + 1, D), np.float32)
    for e in range(N_EXPERTS):
        y_all[e * Cmax:e * Cmax + len(y_list[e])] = y_list[e]

    pos = np.where(pos_of_pair < 0, N_EXPERTS * Cmax, pos_of_pair)
    out_flat = y_all[pos].reshape(N, TOP_K, D).sum(axis=1)
    return out_flat.reshape(Bq, Tq, D).astype(np.float32, copy=False)
